# revision 1
# baseline (speedup 1.0000x reference)
"""Trainium2 Bass kernel for nn_ContextRelation_Module (dense_transformer).

Data-parallel over batch: 8 batches -> 8 NeuronCores, one batch each.

Per-core program (B=1 slice):
  x    [512, 16384]  (C_in, H*W)
  q    = relu(bn(W_q2 @ relu(bn(W_q1 @ x))))            [256, 16384]
  k    = relu(bn(W_k2 @ relu(bn(W_k1 @ ctx))))          [256, 19]
  v    = relu(bn(W_v @ ctx))                            [256, 19]
  simT = k^T @ q * (1/16)                               [19, 16384]
  attT = softmax_j(simT)  (no max-subtract: sim in [0, 0.17])
  ctxT = v^T @ attT                                     [256, 16384]
  y    = relu(bn(W_u @ ctxT))                           [512, 16384]

All big matmuls run with float32r operand tag (fp32 bits, fast PE mode).
BN (inference) is folded host-side into per-channel scale/bias applied by
the ScalarE activation (relu(psum*scale + bias)).

Softmax over the 19-entry partition axis is done with PE helpers:
  denom[1,T]  = ones[19,1]^T @ exp_simT           (partition-dim sum)
  bcast[19,T] = ones[1,19]^T @ recip(denom)       (partition broadcast)
  attT        = exp_simT * bcast                  (DVE)

The per-512-column tile pipeline is software-pipelined across five
emission stages (projections+sim | denom | bcast+normalize | ctx | out)
so PE never waits on ACT/DVE softmax latencies.
"""

import numpy as np

import concourse.bacc as bacc
import concourse.bass as bass
import concourse.mybir as mybir
import concourse.tile as tile
from concourse import bass_utils
from concourse.bass import ts
from concourse.masks import make_identity

AFT = mybir.ActivationFunctionType
F32 = mybir.dt.float32

# problem dims (hardcoded per contract)
B = 8
C = 512            # input/output channels
K = 256            # key_channels
H = 128
W = 128
NCTX = 19          # context tokens
NPIX = H * W       # 16384 pixels per batch
CB = C // 128      # 4 partition blocks of C
KB = K // 128      # 2 partition blocks of K
EPS = 1e-5
SOFTMAX_SCALE = K ** -0.5   # 1/16

# tunables
TN = 512                       # free-dim tile (one PSUM bank of fp32)
MM_DT = mybir.dt.float32r      # matmul operand tag for the big GEMMs
X_DMA = "sync"                 # engine for x-in DMA
Y_DMA = "gpsimd"               # engine for y-out DMA


def _build(npix=NPIX, mm_dt=MM_DT, repeat=1):
    """Build + compile the per-core Bass module.

    repeat>1 re-runs the whole pixel loop (same input/output) for
    differential timing: t(R) - t(1) = (R-1) * T_kernel.
    """
    nt = npix // TN
    MMD = mm_dt  # dtype for tensors feeding the big (fp32r-tagged) matmuls
    nc = bacc.Bacc("TRN2", target_bir_lowering=False, debug=False)

    x_d = nc.dram_tensor("x", [C, npix], MMD, kind="ExternalInput").ap()
    ct_d = nc.dram_tensor("ctxt", [C, NCTX], F32, kind="ExternalInput").ap()
    wq1_d = nc.dram_tensor("wq1", [C, K], MMD, kind="ExternalInput").ap()
    wq2_d = nc.dram_tensor("wq2", [K, K], MMD, kind="ExternalInput").ap()
    wk1_d = nc.dram_tensor("wk1", [C, K], F32, kind="ExternalInput").ap()
    wk2_d = nc.dram_tensor("wk2", [K, K], F32, kind="ExternalInput").ap()
    wv_d = nc.dram_tensor("wv", [C, K], F32, kind="ExternalInput").ap()
    wu_d = nc.dram_tensor("wu", [K, C], MMD, kind="ExternalInput").ap()
    sb_names = ["sq1", "bq1", "sq2", "bq2", "sk1", "bk1", "sk2", "bk2",
                "sv", "bv", "su", "bu"]
    sb_d = {}
    for n in sb_names:
        nblk = CB if n in ("su", "bu") else KB
        sb_d[n] = nc.dram_tensor(n, [128, nblk], F32, kind="ExternalInput").ap()
    y_d = nc.dram_tensor("y", [C, npix], F32, kind="ExternalOutput").ap()

    x_v = x_d.rearrange("(c p) n -> p c n", p=128)
    y_v = y_d.rearrange("(c p) n -> p c n", p=128)

    def mmx(out, lhsT, rhs, start, stop):
        nc.tensor.matmul(out, lhsT, rhs, start=start, stop=stop)

    with tile.TileContext(nc) as tc, nc.allow_low_precision(reason="fp32r matmul operands"):
        with (
            tc.tile_pool(name="consts", bufs=1) as consts,
            tc.tile_pool(name="xin", bufs=4) as xin,
            tc.tile_pool(name="yout", bufs=4) as yout,
            tc.tile_pool(name="work", bufs=2) as work,
            tc.tile_pool(name="psB", bufs=3, space="PSUM") as psB,
            tc.tile_pool(name="psS", bufs=2, space="PSUM") as psS,
        ):
            # ---- constants ----
            wq1_sb = consts.tile([128, CB, K], MMD, name="wq1_sb")
            nc.sync.dma_start(out=wq1_sb, in_=wq1_d.rearrange("(c p) m -> p c m", p=128))
            wq2_sb = consts.tile([128, KB, K], MMD, name="wq2_sb")
            nc.sync.dma_start(out=wq2_sb, in_=wq2_d.rearrange("(c p) m -> p c m", p=128))
            wk1_sb = consts.tile([128, CB, K], F32, name="wk1_sb")
            nc.sync.dma_start(out=wk1_sb, in_=wk1_d.rearrange("(c p) m -> p c m", p=128))
            wk2_sb = consts.tile([128, KB, K], F32, name="wk2_sb")
            nc.sync.dma_start(out=wk2_sb, in_=wk2_d.rearrange("(c p) m -> p c m", p=128))
            wv_sb = consts.tile([128, CB, K], F32, name="wv_sb")
            nc.sync.dma_start(out=wv_sb, in_=wv_d.rearrange("(c p) m -> p c m", p=128))
            wu_sb = consts.tile([128, KB, C], MMD, name="wu_sb")
            nc.sync.dma_start(out=wu_sb, in_=wu_d.rearrange("(c p) m -> p c m", p=128))
            sb = {}
            for n in sb_names:
                nblk = CB if n in ("su", "bu") else KB
                t_ = consts.tile([128, nblk], F32, name=f"{n}_sb")
                nc.sync.dma_start(out=t_, in_=sb_d[n])
                sb[n] = t_
            ct_sb = consts.tile([128, CB, NCTX], F32, name="ct_sb")
            nc.sync.dma_start(out=ct_sb, in_=ct_d.rearrange("(c p) m -> p c m", p=128))

            ones19_f = consts.tile([NCTX, 1], F32, name="ones19_f")
            nc.vector.memset(ones19_f, 1.0)
            ones19 = consts.tile([NCTX, 1], MMD, name="ones19")
            nc.vector.tensor_copy(ones19, ones19_f)
            ones1_f = consts.tile([1, NCTX], F32, name="ones1_f")
            nc.vector.memset(ones1_f, 1.0)
            ones1 = consts.tile([1, NCTX], MMD, name="ones1")
            nc.vector.tensor_copy(ones1, ones1_f)
            ident = consts.tile([128, 128], F32, name="ident")
            make_identity(nc, ident)

            # ---- preamble: k, v projections of the 19-token context (fp32) ----
            k1_sb = consts.tile([128, KB, NCTX], F32, name="k1_sb")
            for m in range(KB):
                p = psB.tile([128, NCTX], F32, tag="mm", name="pk1")
                for c in range(CB):
                    nc.tensor.matmul(p, wk1_sb[:, c, ts(m, 128)], ct_sb[:, c, :],
                                     start=(c == 0), stop=(c == CB - 1))
                nc.scalar.activation(k1_sb[:, m, :], p, AFT.Relu,
                                     bias=sb["bk1"][:, m:m + 1], scale=sb["sk1"][:, m:m + 1])
            k2_sb = consts.tile([128, KB, NCTX], MMD, name="k2_sb")
            for m in range(KB):
                p = psB.tile([128, NCTX], F32, tag="mm", name="pk2")
                for c in range(KB):
                    nc.tensor.matmul(p, wk2_sb[:, c, ts(m, 128)], k1_sb[:, c, :],
                                     start=(c == 0), stop=(c == KB - 1))
                nc.scalar.activation(k2_sb[:, m, :], p, AFT.Relu,
                                     bias=sb["bk2"][:, m:m + 1], scale=sb["sk2"][:, m:m + 1])
            v_sb = consts.tile([128, KB, NCTX], F32, name="v_sb")
            for m in range(KB):
                p = psB.tile([128, NCTX], F32, tag="mm", name="pv")
                for c in range(CB):
                    nc.tensor.matmul(p, wv_sb[:, c, ts(m, 128)], ct_sb[:, c, :],
                                     start=(c == 0), stop=(c == CB - 1))
                nc.scalar.activation(v_sb[:, m, :], p, AFT.Relu,
                                     bias=sb["bv"][:, m:m + 1], scale=sb["sv"][:, m:m + 1])
            # vT [19, KB, 128] via PE transpose
            vT_sb = consts.tile([NCTX, KB, 128], MMD, name="vT_sb")
            for m in range(KB):
                p = psB.tile([NCTX, 128], F32, tag="mm", name="pvt")
                nc.tensor.transpose(p, v_sb[:, m, :], ident)
                nc.vector.tensor_copy(vT_sb[:, m, :], p)

            # ---- main loop, software-pipelined in 3 emission stages ----
            state = {}

            def stageA(t):
                xt = xin.tile([128, CB, TN], MMD, tag="xt", name="xt")
                dma_in = nc.sync if X_DMA == "sync" else nc.gpsimd
                dma_in.dma_start(out=xt, in_=x_v[:, :, ts(t, TN)])
                q1 = work.tile([128, KB, TN], MMD, tag="q1", name="q1")
                for m in range(KB):
                    p = psB.tile([128, TN], F32, tag="mm", name="pq1")
                    for c in range(CB):
                        mmx(p, wq1_sb[:, c, ts(m, 128)], xt[:, c, :],
                            c == 0, c == CB - 1)
                    nc.scalar.activation(q1[:, m, :], p, AFT.Relu,
                                         bias=sb["bq1"][:, m:m + 1], scale=sb["sq1"][:, m:m + 1])
                q2 = work.tile([128, KB, TN], MMD, tag="q2", name="q2")
                for m in range(KB):
                    p = psB.tile([128, TN], F32, tag="mm", name="pq2")
                    for c in range(KB):
                        mmx(p, wq2_sb[:, c, ts(m, 128)], q1[:, c, :],
                            c == 0, c == KB - 1)
                    nc.scalar.activation(q2[:, m, :], p, AFT.Relu,
                                         bias=sb["bq2"][:, m:m + 1], scale=sb["sq2"][:, m:m + 1])
                psim = psS.tile([NCTX, TN], F32, tag="s19", name="psim")
                for c in range(KB):
                    mmx(psim, k2_sb[:, c, :], q2[:, c, :], c == 0, c == KB - 1)
                esim = work.tile([NCTX, TN], MMD, tag="esim", name="esim", bufs=3)
                nc.scalar.activation(esim, psim, AFT.Exp, scale=SOFTMAX_SCALE)
                state[t] = {"esim": esim}

            def stageP(t):
                st = state[t]
                ps1 = psS.tile([1, TN], F32, tag="s1", name="ps1", bufs=1)
                mmx(ps1, ones19, st["esim"], True, True)
                recip = work.tile([1, TN], MMD, tag="recip", name="recip")
                nc.vector.reciprocal(recip, ps1)
                st["recip"] = recip

            def stageQ(t):
                st = state[t]
                pbc = psS.tile([NCTX, TN], F32, tag="s19", name="pbc")
                mmx(pbc, ones1, st["recip"], True, True)
                att = work.tile([NCTX, TN], MMD, tag="att", name="att")
                nc.vector.tensor_mul(att, st["esim"], pbc)
                st["att"] = att

            def stageB1(t):
                st = state[t]
                att = st["att"]
                cxt = work.tile([128, KB, TN], MMD, tag="cxt", name="cxt")
                for m in range(KB):
                    p = psB.tile([128, TN], F32, tag="mm", name="pctx")
                    mmx(p, vT_sb[:, m, :], att, True, True)
                    nc.vector.tensor_copy(cxt[:, m, :], p)
                st["cxt"] = cxt

            def stageB2(t):
                st = state.pop(t)
                cxt = st["cxt"]
                yt = yout.tile([128, CB, TN], F32, tag="yt", name="yt")
                for m in range(CB):
                    p = psB.tile([128, TN], F32, tag="pu", name="pu", bufs=2)
                    for c in range(KB):
                        mmx(p, wu_sb[:, c, ts(m, 128)], cxt[:, c, :],
                            c == 0, c == KB - 1)
                    if m % 2 == 0:
                        nc.scalar.activation(yt[:, m, :], p, AFT.Relu,
                                             bias=sb["bu"][:, m:m + 1], scale=sb["su"][:, m:m + 1])
                    else:
                        nc.vector.tensor_scalar(yt[:, m, :], p, sb["su"][:, m:m + 1],
                                                sb["bu"][:, m:m + 1],
                                                mybir.AluOpType.mult, mybir.AluOpType.add)
                        nc.vector.tensor_scalar_max(yt[:, m, :], yt[:, m, :], 0.0)
                dma_out = nc.sync if Y_DMA == "sync" else nc.gpsimd
                dma_out.dma_start(out=y_v[:, :, ts(t, TN)], in_=yt)

            for r in range(repeat):
                for t in range(nt + 4):
                    if t < nt:
                        stageA(t)
                    if 1 <= t <= nt:
                        stageP(t - 1)
                    if 2 <= t <= nt + 1:
                        stageQ(t - 2)
                    if 3 <= t <= nt + 2:
                        stageB1(t - 3)
                    if t >= 4:
                        stageB2(t - 4)

    nc.compile()
    return nc


def _prepare_inputs(inputs, npix=NPIX):
    """Fold BN, transpose weights, shard over batch. Returns list of in_maps."""
    f = np.float32

    def fold(bn, conv_b):
        g, be, m, v = [np.asarray(a, dtype=np.float64) for a in bn]
        s = g / np.sqrt(v + EPS)
        t = be - m * s
        bias = np.asarray(conv_b, dtype=np.float64) * s + t
        return s.astype(f), bias.astype(f)

    def pack(vec):  # [C'] -> [128, C'//128], channel = blk*128 + p
        return np.ascontiguousarray(np.asarray(vec, f).reshape(-1, 128).T)

    sq1, bq1 = fold(inputs["qbn1"], inputs["qb1"])
    sq2, bq2 = fold(inputs["qbn2"], inputs["qb2"])
    sk1, bk1 = fold(inputs["kbn1"], inputs["kb1"])
    sk2, bk2 = fold(inputs["kbn2"], inputs["kb2"])
    sv, bv = fold(inputs["vbn"], inputs["vb"])
    su, bu = fold(inputs["ubn"], inputs["ub"])

    base = {
        "wq1": np.ascontiguousarray(np.asarray(inputs["qW1"], f).T),
        "wq2": np.ascontiguousarray(np.asarray(inputs["qW2"], f).T),
        "wk1": np.ascontiguousarray(np.asarray(inputs["kW1"], f).T),
        "wk2": np.ascontiguousarray(np.asarray(inputs["kW2"], f).T),
        "wv": np.ascontiguousarray(np.asarray(inputs["vW"], f).T),
        "wu": np.ascontiguousarray(np.asarray(inputs["uW"], f).T),
        "sq1": pack(sq1), "bq1": pack(bq1), "sq2": pack(sq2), "bq2": pack(bq2),
        "sk1": pack(sk1), "bk1": pack(bk1), "sk2": pack(sk2), "bk2": pack(bk2),
        "sv": pack(sv), "bv": pack(bv), "su": pack(su), "bu": pack(bu),
    }
    x = np.asarray(inputs["x"], f)
    ctx = np.asarray(inputs["context"], f)
    in_maps = []
    for b_i in range(x.shape[0]):
        m = dict(base)
        m["x"] = np.ascontiguousarray(x[b_i].reshape(C, -1)[:, :npix])
        m["ctxt"] = np.ascontiguousarray(ctx[b_i].reshape(C, NCTX))
        in_maps.append(m)
    return in_maps


_NC_CACHE = {}


def _get_nc(npix=NPIX):
    key = (npix, str(MM_DT), TN)
    if key not in _NC_CACHE:
        _NC_CACHE[key] = _build(npix)
    return _NC_CACHE[key]


def run(inputs, trace=False, **kwargs):
    """Run on 8 cores; returns (y [8,512,128,128], BassKernelResults)."""
    nc = _get_nc()
    in_maps = _prepare_inputs(inputs)
    res = bass_utils.run_bass_kernel_spmd(
        nc, in_maps, core_ids=list(range(B)), trace=trace, **kwargs)
    y = np.stack([res.results[b]["y"].reshape(C, H, W) for b in range(B)])
    return y.astype(np.float32), res


def kernel(**inputs):
    y, _ = run(inputs)
    return y

